# revision 20
# baseline (speedup 1.0000x reference)
"""Trainium2 Bass kernel for MinibatchDiscrimination — symmetric j-window.

Reference computation (fp32):
    m = (x @ W.T + b).reshape(nb, 64, 16)            # nb=512
    d[i,j,B] = sum_c |m[i,B,c] - m[j,B,c]|
    o[i,B]   = sum_j exp(-d[i,j,B])
    out      = concat(x, o, axis=1)                   # (512, 1088)

E = exp(-d) is symmetric, so each unordered block pair only needs to be
computed once.  Core c owns global row block c (local rows 0..63) and a
j-window of 5 blocks (local j 0..319 = global blocks c..c+4).  Block
pairs at cyclic gap 1..3 are covered by the lower core, gap 4 by cores
0..3 only — cores 4..7 receive junk rows for local j 256..319 built so
that every feature-sum S_junk ~ JUNK_K, which drives exp(-psd) below
the fp32 underflow threshold: those E columns are exactly 0.0 and can
flow through the accumulations unconditionally.  Each core emits:
    oA[B, i] = sum_{j<320} E(i, j)        (exp accum_out row sums)
    oc[B, j-64] = sum_i E(i, j), j in [64, 320)  (column sums via
        identity-matmul accumulation into PSUM)
The host adds row parts and column parts into the full o; the junk
columns of cores 4..7 contribute zeros everywhere.

Everything else (fp8 DoubleRow projection, |a-b| = a+b-2min algebra
with the S seed matmul, exact self term, exp accum_out) is as in the
non-symmetric kernel; see the docstring history in git.  The container's
walrus only allows single-op tensor_scalar on DVE/POOL and requires
DoubleRow matmuls to write at PSUM partition base 0 (hence one PSUM
tile per output row).
"""

import sys
import numpy as np

if "/opt/trn_rl_repo" not in sys.path:
    sys.path.insert(0, "/opt/trn_rl_repo")

NB = 512          # batch rows
NIN = 1024        # n_in
NBF = 64          # n_B
NCD = 16          # n_C
FOUT = NBF * NCD  # 1024 projection features
NCORES = 8
IB = NB // NCORES  # 64 output rows per core
JW = 5 * IB        # 320-column local j window
JA = 4 * IB        # row-sum A range [0, 256)
WSCALE = 64.0      # host multiplies W by this; psum copy divides it out

F8_TILES = (0, 1, 2, 3)   # fp8 min-path (DoubleRow matmuls)
F16_TILES = (4, 5, 6, 7)  # f16 min-path (DVE 4x mode)

# engine per (i%2, tile) for the fp8 min ops ('V' DVE / 'P' POOL)
MIN_ENG = {
    (0, 0): "V", (0, 1): "P", (0, 2): "P", (0, 3): "P",
    (1, 0): "P", (1, 1): "P", (1, 2): "P", (1, 3): "V",
}
JUNK_K = 230.0     # scale for the junk rows (cores 4-7): projects every
                   # feature-sum S to ~230 so exp(-psd) underflows to 0

_CACHE = {}


def _build_program():
    import concourse.bass as bass
    import concourse.tile as tile
    from concourse import mybir
    from contextlib import ExitStack

    f32 = mybir.dt.float32
    f16 = mybir.dt.float16
    f8 = mybir.dt.float8e4
    Alu = mybir.AluOpType
    Act = mybir.ActivationFunctionType
    DR = mybir.MatmulPerfMode.DoubleRow

    nc = bass.Bass()
    x8_d = nc.declare_dram_parameter("x8", [128, 4 * 2 * JW], f8, isOutput=False)
    w8_d = nc.declare_dram_parameter("w8", [128, 8 * 4 * 2 * 128], f8, isOutput=False)
    i8_d = nc.declare_dram_parameter("i8", [128, 2 * 2 * NBF], f8, isOutput=False)
    i16_d = nc.declare_dram_parameter("i16", [128, 4 * NBF], f16, isOutput=False)
    iS8_d = nc.declare_dram_parameter("iS8", [128, 4 * NBF], f8, isOutput=False)
    iS16_d = nc.declare_dram_parameter("iS16", [128, 4 * NBF], f16, isOutput=False)
    stkI_d = nc.declare_dram_parameter("stkI", [NBF, 128], f16, isOutput=False)
    b_d = nc.declare_dram_parameter("b", [FOUT], f32, isOutput=False)
    oA_d = nc.declare_dram_parameter("oA", [NBF, IB], f32, isOutput=True)
    oc_d = nc.declare_dram_parameter("oc", [NBF, JW - IB], f32, isOutput=True)

    with tile.TileContext(nc) as tc, ExitStack() as ctx:
        singles = ctx.enter_context(tc.tile_pool(name="singles", bufs=1))
        scratch = ctx.enter_context(tc.tile_pool(name="scratch", bufs=12))
        epool = ctx.enter_context(tc.tile_pool(name="epool", bufs=4))
        psA = ctx.enter_context(tc.tile_pool(name="psA", bufs=2, space="PSUM"))
        psS = ctx.enter_context(tc.tile_pool(name="psS", bufs=1, space="PSUM"))
        psC = ctx.enter_context(tc.tile_pool(name="psC", bufs=1, space="PSUM"))
        psB = ctx.enter_context(tc.tile_pool(name="psB", bufs=4, space="PSUM"))

        dma = nc.default_dma_engine

        # ---- persistent loads -------------------------------------------
        x8 = singles.tile([128, 4, 2, JW], f8, name="x8", tag="x8")
        dma.dma_start(out=x8, in_=x8_d.rearrange("p (q m j) -> p q m j", q=4, m=2))
        w8 = singles.tile([128, 8, 4, 2, 128], f8, name="w8", tag="w8")
        dma.dma_start(
            out=w8, in_=w8_d.rearrange("p (t q m f) -> p t q m f", t=8, q=4, m=2)
        )
        i8 = singles.tile([128, 2, 2, NBF], f8, name="i8", tag="i8")
        dma.dma_start(out=i8, in_=i8_d.rearrange("p (q m b) -> p q m b", q=2, m=2))
        i16 = singles.tile([128, 4, NBF], f16, name="i16", tag="i16")
        dma.dma_start(out=i16, in_=i16_d.rearrange("p (t b) -> p t b", t=4))
        iS8 = singles.tile([128, 4, NBF], f8, name="iS8", tag="iS8")
        dma.dma_start(out=iS8, in_=iS8_d.rearrange("p (t b) -> p t b", t=4))
        iS16 = singles.tile([128, 4, NBF], f16, name="iS16", tag="iS16")
        dma.dma_start(out=iS16, in_=iS16_d.rearrange("p (t b) -> p t b", t=4))
        stkI = singles.tile([NBF, 128], f16, name="stkI", tag="stkI")
        dma.dma_start(out=stkI, in_=stkI_d[:, :])
        b_sb = singles.tile([128, 8], f32, name="b_sb", tag="b_sb")
        dma.dma_start(out=b_sb, in_=b_d.rearrange("(t p) -> p t", p=128))

        # ---- mT = (x @ W.T)/WSCALE + b  via fp8 DoubleRow ---------------
        mT = [None] * 8
        mC = [None] * 8
        for t in range(8):
            ps = psA.tile([128, JW], f32, name="mps", tag="mps")
            for q in range(4):
                nc.tensor.matmul(
                    ps, lhsT=w8[:, t, q], rhs=x8[:, q],
                    perf_mode=DR, start=(q == 0), stop=(q == 3),
                )
            mt = singles.tile([128, JW], f16, name=f"mT{t}", tag=f"mT{t}")
            nc.scalar.activation(
                out=mt, in_=ps, func=Act.Identity,
                bias=b_sb[:, t : t + 1], scale=1.0 / WSCALE,
            )
            mT[t] = mt
            # f32 upcast of the 64 local columns (scalar operands)
            mc = singles.tile([128, IB], f32, name=f"mC{t}", tag=f"mC{t}")
            nc.vector.tensor_copy(mc, mt[:, 0:IB])
            mC[t] = mc

        # fp8 copies of the fp8-path tiles (S must sum what min will emit)
        m8 = {}
        for t in F8_TILES:
            c8 = singles.tile([128, JW], f8, name=f"m8_{t}", tag=f"m8_{t}")
            nc.vector.tensor_copy(c8, mT[t])
            m8[t] = c8

        # ---- S = sum_c m over min-path tiles  (psum, f32-exact) ---------
        pS = psS.tile([NBF, JW], f32, name="pS", tag="pS")
        for t in F8_TILES:
            nc.tensor.matmul(pS, lhsT=iS8[:, t], rhs=m8[t],
                             start=(t == 0), stop=False)
        for t in F16_TILES:
            nc.tensor.matmul(pS, lhsT=iS16[:, t - 4], rhs=mT[t],
                             start=False, stop=(t == 7))

        negS = singles.tile([NBF, JW], f16, name="negS", tag="negS")
        nc.scalar.activation(out=negS, in_=pS, func=Act.Copy, bias=0.0, scale=-1.0)
        bias64 = singles.tile([NBF, IB], f32, name="bias64", tag="bias64")
        nc.vector.scalar_tensor_tensor(
            out=bias64, in0=pS[:, 0:IB], scalar=-2.0, in1=negS[:, 0:IB],
            op0=Alu.mult, op1=Alu.subtract,
        )

        oA = singles.tile([NBF, IB], f32, name="oA", tag="oA")
        pC = psC.tile([NBF, JW - IB], f32, name="pC", tag="pC")

        # ---- pairwise loop, one row per PSUM tile -----------------------
        for i in range(IB):
            psd = psB.tile([NBF, JW], f32, name="psd", tag="psd")
            nc.tensor.matmul(psd, lhsT=stkI[:, 0:NBF], rhs=negS,
                             start=True, stop=False)
            for q in range(2):
                ab8 = scratch.tile([128, 2, JW], f8, name="ab", tag="ab")
                for mm in range(2):
                    t = 2 * q + mm
                    e = (nc.vector if MIN_ENG[(i % 2, t)] == "V"
                         else nc.gpsimd)
                    e.tensor_scalar_min(ab8[:, mm], mT[t], mC[t][:, i : i + 1])
                nc.tensor.matmul(
                    psd, lhsT=i8[:, q], rhs=ab8,
                    perf_mode=DR, start=False, stop=False,
                )
            for t in F16_TILES:
                ab16 = scratch.tile([128, JW], f16, name="ab16", tag="ab16")
                nc.vector.tensor_scalar_min(ab16, mT[t], mC[t][:, i : i + 1])
                nc.tensor.matmul(
                    psd, lhsT=i16[:, t - 4], rhs=ab16,
                    start=False, stop=(t == 7),
                )
            E = epool.tile([NBF, JW], f16, name="E", tag="E")
            nc.scalar.activation(
                out=E, in_=psd, func=Act.Exp,
                bias=bias64[:, i : i + 1], scale=1.0,
                accum_out=oA[:, i : i + 1],
            )
            # column sums over j in [64, 320): identity-matmul accumulate
            nc.tensor.matmul(pC, lhsT=stkI[:, 0:NBF], rhs=E[:, IB:JW],
                             start=(i == 0), stop=(i == IB - 1))

        ocs = singles.tile([NBF, JW - IB], f32, name="ocs", tag="ocs")
        nc.scalar.activation(out=ocs, in_=pC, func=Act.Copy, bias=0.0, scale=1.0)

        dma.dma_start(out=oA_d[:, :], in_=oA)
        dma.dma_start(out=oc_d[:, :], in_=ocs)

    _split_multi_waits(nc, mybir)
    return nc


def _split_multi_waits(nc, mybir):
    """Hoist multi-waits onto single-wait NoOps (walrus limitation)."""
    f = nc.m.functions[0]
    n_split = 0
    for blk in f.blocks:
        idx = 0
        while idx < len(blk.instructions):
            inst = blk.instructions[idx]
            si = inst.sync_info
            waits = list(si.on_wait) if si is not None and si.on_wait else []
            if len(waits) > 1:
                bysem = {}
                for w in waits:
                    k = w.id
                    if k not in bysem or (w.wait_value or 0) > (
                        bysem[k].wait_value or 0
                    ):
                        bysem[k] = w
                waits = list(bysem.values())
                for w in waits[:-1]:
                    nop = mybir.InstNoOp(
                        name=nc.get_next_instruction_name(), ins=[], outs=[]
                    )
                    nop.engine = inst.engine
                    nop.sync_info = mybir.SyncInfo(on_wait=[w], on_update=[])
                    blk.instructions.insert(idx, nop)
                    idx += 1
                    n_split += 1
                si.on_wait = [waits[-1]]
            idx += 1
    return n_split


def _get_program():
    if "nc" not in _CACHE:
        _CACHE["nc"] = _build_program()
    return _CACHE["nc"]


def make_in_maps(x, W, b):
    import ml_dtypes

    f8 = ml_dtypes.float8_e4m3
    x = np.ascontiguousarray(x, dtype=np.float32)
    W = np.ascontiguousarray(W, dtype=np.float32)
    b = np.ascontiguousarray(b, dtype=np.float32)

    wT = np.ascontiguousarray(W.T * WSCALE).astype(f8)          # [1024 k, 1024 f]
    w8 = wT.reshape(4, 2, 128, 8, 128)                           # [q, m, p, t, f]
    w8 = np.ascontiguousarray(w8.transpose(2, 3, 0, 1, 4)).reshape(128, -1)

    ind = np.zeros((8, 128, NBF), dtype=np.float32)
    ch = np.arange(FOUT).reshape(8, 128)
    t_, p_ = np.meshgrid(np.arange(8), np.arange(128), indexing="ij")
    ind[t_, p_, ch // NCD] = 1.0
    i8 = np.ascontiguousarray(
        (2.0 * ind[:4]).reshape(2, 2, 128, NBF).transpose(2, 0, 1, 3)
    ).astype(f8).reshape(128, -1)
    i16 = np.ascontiguousarray(
        (2.0 * ind[4:]).transpose(1, 0, 2)
    ).astype(np.float16).reshape(128, -1)
    iS8 = np.ascontiguousarray(ind[:4].transpose(1, 0, 2)).astype(f8).reshape(128, -1)
    iS16 = np.ascontiguousarray(ind[4:].transpose(1, 0, 2)).astype(np.float16).reshape(128, -1)
    stkI = np.concatenate([np.eye(NBF), np.eye(NBF)], axis=1).astype(np.float16)

    U = W.reshape(NBF, NCD, NIN).sum(1)
    xs = (U.T @ np.linalg.solve(U @ U.T, np.full(NBF, 1000.0))).astype(np.float32)
    xs *= JUNK_K / np.abs(xs).max()

    in_maps = []
    for c in range(NCORES):
        xr = np.roll(x, -IB * c, axis=0)[0:JW].copy()            # [320, 1024]
        if c >= 4:
            xr[JA:JW] = xs[None, :]   # junk rows: E == 0 exactly
        xT = np.ascontiguousarray(xr.T).astype(f8)               # [1024 k, 320 j]
        x8 = xT.reshape(4, 2, 128, JW)                            # [q, m, p, j]
        x8 = np.ascontiguousarray(x8.transpose(2, 0, 1, 3)).reshape(128, -1)
        in_maps.append({
            "x8": x8, "w8": w8, "i8": i8, "i16": i16,
            "iS8": iS8, "iS16": iS16, "stkI": stkI, "b": b,
        })
    return in_maps


def assemble_o(results):
    """results[c] -> dict with 'oA' [64,64], 'oc' [64,256]."""
    o_full = np.zeros((NB, NBF), dtype=np.float64)
    for c in range(NCORES):
        o_full[IB * c : IB * (c + 1)] += np.asarray(results[c]["oA"],
                                                    dtype=np.float64).T
        oc = np.asarray(results[c]["oc"], dtype=np.float64)
        g = (IB * c + IB + np.arange(JW - IB)) % NB
        o_full[g] += oc.T
    return o_full.astype(np.float32)


def kernel(x, W, b):
    from concourse.bass_utils import run_bass_kernel_spmd

    x = np.ascontiguousarray(x, dtype=np.float32)
    nc = _get_program()
    in_maps = make_in_maps(x, W, b)

    res = run_bass_kernel_spmd(nc, in_maps, list(range(NCORES)), trace=False)
    _CACHE["last_results"] = res

    o_full = assemble_o(res.results)
    return np.concatenate([x, o_full], axis=1)


# revision 21
# speedup vs baseline: 1.0978x; 1.0978x over previous
"""Trainium2 Bass kernel for MinibatchDiscrimination — symmetric j-window.

Reference computation (fp32):
    m = (x @ W.T + b).reshape(nb, 64, 16)            # nb=512
    d[i,j,B] = sum_c |m[i,B,c] - m[j,B,c]|
    o[i,B]   = sum_j exp(-d[i,j,B])
    out      = concat(x, o, axis=1)                   # (512, 1088)

E = exp(-d) is symmetric, so each unordered block pair only needs to be
computed once.  Core c owns global row block c (local rows 0..63) and a
j-window of 5 blocks (local j 0..319 = global blocks c..c+4).  Block
pairs at cyclic gap 1..3 are covered by the lower core, gap 4 by cores
0..3 only — cores 4..7 receive junk rows for local j 256..319 built so
that every feature-sum S_junk ~ JUNK_K, which drives exp(-psd) below
the fp32 underflow threshold: those E columns are exactly 0.0 and can
flow through the accumulations unconditionally.  Each core emits:
    oA[B, i] = sum_{j<320} E(i, j)        (exp accum_out row sums)
    oc[B, j-64] = sum_i E(i, j), j in [64, 320)  (column sums via
        identity-matmul accumulation into PSUM)
The host adds row parts and column parts into the full o; the junk
columns of cores 4..7 contribute zeros everywhere.

Everything else (fp8 DoubleRow projection, |a-b| = a+b-2min algebra
with the S seed matmul, exact self term, exp accum_out) is as in the
non-symmetric kernel; see the docstring history in git.  The container's
walrus only allows single-op tensor_scalar on DVE/POOL and requires
DoubleRow matmuls to write at PSUM partition base 0 (hence one PSUM
tile per output row).
"""

import sys
import numpy as np

if "/opt/trn_rl_repo" not in sys.path:
    sys.path.insert(0, "/opt/trn_rl_repo")

NB = 512          # batch rows
NIN = 1024        # n_in
NBF = 64          # n_B
NCD = 16          # n_C
FOUT = NBF * NCD  # 1024 projection features
NCORES = 8
IB = NB // NCORES  # 64 output rows per core
JW = 5 * IB        # 320-column local j window
JA = 4 * IB        # row-sum A range [0, 256)
WSCALE = 64.0      # host multiplies W by this; psum copy divides it out

F8_TILES = (0, 1, 2, 3)   # fp8 min-path (DoubleRow matmuls)
F16_TILES = (4, 5, 6, 7)  # f16 min-path (DVE 4x mode)

# engine per (i%2, tile) for the fp8 min ops ('V' DVE / 'P' POOL)
MIN_ENG = {
    (0, 0): "V", (0, 1): "P", (0, 2): "P", (0, 3): "P",
    (1, 0): "P", (1, 1): "P", (1, 2): "P", (1, 3): "V",
}
JUNK_K = 230.0     # scale for the junk rows (cores 4-7): projects every
                   # feature-sum S to ~230 so exp(-psd) underflows to 0

_CACHE = {}


def _build_program():
    import concourse.bass as bass
    import concourse.tile as tile
    from concourse import mybir
    from contextlib import ExitStack

    f32 = mybir.dt.float32
    f16 = mybir.dt.float16
    f8 = mybir.dt.float8e4
    Alu = mybir.AluOpType
    Act = mybir.ActivationFunctionType
    DR = mybir.MatmulPerfMode.DoubleRow

    nc = bass.Bass()
    x8_d = nc.declare_dram_parameter("x8", [128, 4 * 2 * JW], f8, isOutput=False)
    w8_d = nc.declare_dram_parameter("w8", [128, 8 * 4 * 2 * 128], f8, isOutput=False)
    i8_d = nc.declare_dram_parameter("i8", [128, 2 * 2 * NBF], f8, isOutput=False)
    i16_d = nc.declare_dram_parameter("i16", [128, 4 * NBF], f16, isOutput=False)
    iS8_d = nc.declare_dram_parameter("iS8", [128, 4 * NBF], f8, isOutput=False)
    iS16_d = nc.declare_dram_parameter("iS16", [128, 4 * NBF], f16, isOutput=False)
    stkI_d = nc.declare_dram_parameter("stkI", [NBF, 128], f16, isOutput=False)
    iC8_d = nc.declare_dram_parameter("iC8", [NBF, 2 * NBF], f8, isOutput=False)
    b_d = nc.declare_dram_parameter("b", [FOUT], f32, isOutput=False)
    oA_d = nc.declare_dram_parameter("oA", [NBF, IB], f32, isOutput=True)
    oc_d = nc.declare_dram_parameter("oc", [NBF, JW - IB], f32, isOutput=True)

    with tile.TileContext(nc) as tc, ExitStack() as ctx:
        singles = ctx.enter_context(tc.tile_pool(name="singles", bufs=1))
        scratch = ctx.enter_context(tc.tile_pool(name="scratch", bufs=16))
        epool = ctx.enter_context(tc.tile_pool(name="epool", bufs=4))
        psA = ctx.enter_context(tc.tile_pool(name="psA", bufs=2, space="PSUM"))
        psS = ctx.enter_context(tc.tile_pool(name="psS", bufs=1, space="PSUM"))
        psC = ctx.enter_context(tc.tile_pool(name="psC", bufs=1, space="PSUM"))
        psB = ctx.enter_context(tc.tile_pool(name="psB", bufs=4, space="PSUM"))

        dma = nc.default_dma_engine

        # ---- persistent loads -------------------------------------------
        x8 = singles.tile([128, 4, 2, JW], f8, name="x8", tag="x8")
        x8_r = x8_d.rearrange("p (q m j) -> p q m j", q=4, m=2)
        for q in range(4):
            dma.dma_start(out=x8[:, q], in_=x8_r[:, q])
        w8 = singles.tile([128, 8, 4, 2, 128], f8, name="w8", tag="w8")
        w8_r = w8_d.rearrange("p (t q m f) -> p t q m f", t=8, q=4, m=2)
        for t in range(8):
            dma.dma_start(out=w8[:, t], in_=w8_r[:, t])
        i8 = singles.tile([128, 2, 2, NBF], f8, name="i8", tag="i8")
        dma.dma_start(out=i8, in_=i8_d.rearrange("p (q m b) -> p q m b", q=2, m=2))
        i16 = singles.tile([128, 4, NBF], f16, name="i16", tag="i16")
        dma.dma_start(out=i16, in_=i16_d.rearrange("p (t b) -> p t b", t=4))
        iS8 = singles.tile([128, 4, NBF], f8, name="iS8", tag="iS8")
        dma.dma_start(out=iS8, in_=iS8_d.rearrange("p (t b) -> p t b", t=4))
        iS16 = singles.tile([128, 4, NBF], f16, name="iS16", tag="iS16")
        dma.dma_start(out=iS16, in_=iS16_d.rearrange("p (t b) -> p t b", t=4))
        stkI = singles.tile([NBF, 128], f16, name="stkI", tag="stkI")
        dma.dma_start(out=stkI, in_=stkI_d[:, :])
        iC8 = singles.tile([NBF, 2, NBF], f8, name="iC8", tag="iC8")
        dma.dma_start(out=iC8, in_=iC8_d.rearrange("p (m b) -> p m b", m=2))
        b_sb = singles.tile([128, 8], f32, name="b_sb", tag="b_sb")
        dma.dma_start(out=b_sb, in_=b_d.rearrange("(t p) -> p t", p=128))

        # ---- mT = (x @ W.T)/WSCALE + b  via fp8 DoubleRow ---------------
        mT = [None] * 8
        mC = [None] * 8
        for t in range(8):
            ps = psA.tile([128, JW], f32, name="mps", tag="mps")
            for q in range(4):
                nc.tensor.matmul(
                    ps, lhsT=w8[:, t, q], rhs=x8[:, q],
                    perf_mode=DR, start=(q == 0), stop=(q == 3),
                )
            mt = singles.tile([128, JW], f16, name=f"mT{t}", tag=f"mT{t}")
            nc.scalar.activation(
                out=mt, in_=ps, func=Act.Identity,
                bias=b_sb[:, t : t + 1], scale=1.0 / WSCALE,
            )
            mT[t] = mt
            # f32 upcast of the 64 local columns (scalar operands)
            mc = singles.tile([128, IB], f32, name=f"mC{t}", tag=f"mC{t}")
            nc.vector.tensor_copy(mc, mt[:, 0:IB])
            mC[t] = mc

        # fp8 copies of the fp8-path tiles (S must sum what min will emit)
        m8 = {}
        for t in F8_TILES:
            c8 = singles.tile([128, JW], f8, name=f"m8_{t}", tag=f"m8_{t}")
            nc.vector.tensor_copy(c8, mT[t])
            m8[t] = c8

        # ---- S = sum_c m over min-path tiles  (psum, f32-exact) ---------
        pS = psS.tile([NBF, JW], f32, name="pS", tag="pS")
        for t in F8_TILES:
            nc.tensor.matmul(pS, lhsT=iS8[:, t], rhs=m8[t],
                             start=(t == 0), stop=False)
        for t in F16_TILES:
            nc.tensor.matmul(pS, lhsT=iS16[:, t - 4], rhs=mT[t],
                             start=False, stop=(t == 7))

        negS = singles.tile([NBF, JW], f16, name="negS", tag="negS")
        nc.scalar.activation(out=negS, in_=pS, func=Act.Copy, bias=0.0, scale=-1.0)
        bias64 = singles.tile([NBF, IB], f32, name="bias64", tag="bias64")
        nc.vector.scalar_tensor_tensor(
            out=bias64, in0=pS[:, 0:IB], scalar=-2.0, in1=negS[:, 0:IB],
            op0=Alu.mult, op1=Alu.subtract,
        )

        oA = singles.tile([NBF, IB], f32, name="oA", tag="oA")
        pC = psC.tile([NBF, JW - IB], f32, name="pC", tag="pC")

        # ---- pairwise loop, one row per PSUM tile -----------------------
        E2 = None
        for i in range(IB):
            psd = psB.tile([NBF, JW], f32, name="psd", tag="psd")
            if i % 2 == 0:
                E2 = epool.tile([NBF, 2, JW], f8, name="E2", tag="E2")
            nc.tensor.matmul(psd, lhsT=stkI[:, 0:NBF], rhs=negS,
                             start=True, stop=False)
            for q in range(2):
                ab8 = scratch.tile([128, 2, JW], f8, name="ab", tag="ab")
                for mm in range(2):
                    t = 2 * q + mm
                    e = (nc.vector if MIN_ENG[(i % 2, t)] == "V"
                         else nc.gpsimd)
                    e.tensor_scalar_min(ab8[:, mm], mT[t], mC[t][:, i : i + 1])
                nc.tensor.matmul(
                    psd, lhsT=i8[:, q], rhs=ab8,
                    perf_mode=DR, start=False, stop=False,
                )
            for t in F16_TILES:
                ab16 = scratch.tile([128, JW], f16, name="ab16", tag="ab16")
                nc.vector.tensor_scalar_min(ab16, mT[t], mC[t][:, i : i + 1])
                nc.tensor.matmul(
                    psd, lhsT=i16[:, t - 4], rhs=ab16,
                    start=False, stop=(t == 7),
                )
            nc.scalar.activation(
                out=E2[:, i % 2], in_=psd, func=Act.Exp,
                bias=bias64[:, i : i + 1], scale=1.0,
                accum_out=oA[:, i : i + 1],
            )
            if i % 2 == 1:
                # column sums over j in [64, 320): DoubleRow accumulates
                # both rows' E at once
                nc.tensor.matmul(pC, lhsT=iC8, rhs=E2[:, :, IB:JW],
                                 perf_mode=DR,
                                 start=(i == 1), stop=(i == IB - 1))

        ocs = singles.tile([NBF, JW - IB], f32, name="ocs", tag="ocs")
        nc.scalar.activation(out=ocs, in_=pC, func=Act.Copy, bias=0.0, scale=1.0)

        dma.dma_start(out=oA_d[:, :], in_=oA)
        dma.dma_start(out=oc_d[:, :], in_=ocs)

    _split_multi_waits(nc, mybir)
    return nc


def _split_multi_waits(nc, mybir):
    """Hoist multi-waits onto single-wait NoOps (walrus limitation)."""
    f = nc.m.functions[0]
    n_split = 0
    for blk in f.blocks:
        idx = 0
        while idx < len(blk.instructions):
            inst = blk.instructions[idx]
            si = inst.sync_info
            waits = list(si.on_wait) if si is not None and si.on_wait else []
            if len(waits) > 1:
                bysem = {}
                for w in waits:
                    k = w.id
                    if k not in bysem or (w.wait_value or 0) > (
                        bysem[k].wait_value or 0
                    ):
                        bysem[k] = w
                waits = list(bysem.values())
                for w in waits[:-1]:
                    nop = mybir.InstNoOp(
                        name=nc.get_next_instruction_name(), ins=[], outs=[]
                    )
                    nop.engine = inst.engine
                    nop.sync_info = mybir.SyncInfo(on_wait=[w], on_update=[])
                    blk.instructions.insert(idx, nop)
                    idx += 1
                    n_split += 1
                si.on_wait = [waits[-1]]
            idx += 1
    return n_split


def _get_program():
    if "nc" not in _CACHE:
        _CACHE["nc"] = _build_program()
    return _CACHE["nc"]


def make_in_maps(x, W, b):
    import ml_dtypes

    f8 = ml_dtypes.float8_e4m3
    x = np.ascontiguousarray(x, dtype=np.float32)
    W = np.ascontiguousarray(W, dtype=np.float32)
    b = np.ascontiguousarray(b, dtype=np.float32)

    wT = np.ascontiguousarray(W.T * WSCALE).astype(f8)          # [1024 k, 1024 f]
    w8 = wT.reshape(4, 2, 128, 8, 128)                           # [q, m, p, t, f]
    w8 = np.ascontiguousarray(w8.transpose(2, 3, 0, 1, 4)).reshape(128, -1)

    ind = np.zeros((8, 128, NBF), dtype=np.float32)
    ch = np.arange(FOUT).reshape(8, 128)
    t_, p_ = np.meshgrid(np.arange(8), np.arange(128), indexing="ij")
    ind[t_, p_, ch // NCD] = 1.0
    i8 = np.ascontiguousarray(
        (2.0 * ind[:4]).reshape(2, 2, 128, NBF).transpose(2, 0, 1, 3)
    ).astype(f8).reshape(128, -1)
    i16 = np.ascontiguousarray(
        (2.0 * ind[4:]).transpose(1, 0, 2)
    ).astype(np.float16).reshape(128, -1)
    iS8 = np.ascontiguousarray(ind[:4].transpose(1, 0, 2)).astype(f8).reshape(128, -1)
    iS16 = np.ascontiguousarray(ind[4:].transpose(1, 0, 2)).astype(np.float16).reshape(128, -1)
    stkI = np.concatenate([np.eye(NBF), np.eye(NBF)], axis=1).astype(np.float16)
    iC8 = np.concatenate([np.eye(NBF), np.eye(NBF)], axis=1).astype(f8)

    U = W.reshape(NBF, NCD, NIN).sum(1)
    xs = (U.T @ np.linalg.solve(U @ U.T, np.full(NBF, 1000.0))).astype(np.float32)
    xs *= JUNK_K / np.abs(xs).max()

    in_maps = []
    for c in range(NCORES):
        xr = np.roll(x, -IB * c, axis=0)[0:JW].copy()            # [320, 1024]
        if c >= 4:
            xr[JA:JW] = xs[None, :]   # junk rows: E == 0 exactly
        xT = np.ascontiguousarray(xr.T).astype(f8)               # [1024 k, 320 j]
        x8 = xT.reshape(4, 2, 128, JW)                            # [q, m, p, j]
        x8 = np.ascontiguousarray(x8.transpose(2, 0, 1, 3)).reshape(128, -1)
        in_maps.append({
            "x8": x8, "w8": w8, "i8": i8, "i16": i16,
            "iS8": iS8, "iS16": iS16, "stkI": stkI, "iC8": iC8, "b": b,
        })
    return in_maps


def assemble_o(results):
    """results[c] -> dict with 'oA' [64,64], 'oc' [64,256]."""
    o_full = np.zeros((NB, NBF), dtype=np.float64)
    for c in range(NCORES):
        o_full[IB * c : IB * (c + 1)] += np.asarray(results[c]["oA"],
                                                    dtype=np.float64).T
        oc = np.asarray(results[c]["oc"], dtype=np.float64)
        g = (IB * c + IB + np.arange(JW - IB)) % NB
        o_full[g] += oc.T
    return o_full.astype(np.float32)


def kernel(x, W, b):
    from concourse.bass_utils import run_bass_kernel_spmd

    x = np.ascontiguousarray(x, dtype=np.float32)
    nc = _get_program()
    in_maps = make_in_maps(x, W, b)

    res = run_bass_kernel_spmd(nc, in_maps, list(range(NCORES)), trace=False)
    _CACHE["last_results"] = res

    o_full = assemble_o(res.results)
    return np.concatenate([x, o_full], axis=1)


# revision 22
# speedup vs baseline: 1.1867x; 1.0810x over previous
"""Trainium2 Bass kernel for MinibatchDiscrimination — symmetric j-window.

Reference computation (fp32):
    m = (x @ W.T + b).reshape(nb, 64, 16)            # nb=512
    d[i,j,B] = sum_c |m[i,B,c] - m[j,B,c]|
    o[i,B]   = sum_j exp(-d[i,j,B])
    out      = concat(x, o, axis=1)                   # (512, 1088)

E = exp(-d) is symmetric, so each unordered block pair only needs to be
computed once.  Core c owns global row block c (local rows 0..63) and a
j-window of 5 blocks (local j 0..319 = global blocks c..c+4).  Block
pairs at cyclic gap 1..3 are covered by the lower core, gap 4 by cores
0..3 only — cores 4..7 receive junk rows for local j 256..319 built so
that every feature-sum S_junk ~ JUNK_K, which drives exp(-psd) below
the fp32 underflow threshold: those E columns are exactly 0.0 and can
flow through the accumulations unconditionally.  Each core emits:
    oA[B, i] = sum_{j<320} E(i, j)        (exp accum_out row sums)
    oc[B, j-64] = sum_i E(i, j), j in [64, 320)  (column sums via
        identity-matmul accumulation into PSUM)
The host adds row parts and column parts into the full o; the junk
columns of cores 4..7 contribute zeros everywhere.

Everything else (fp8 DoubleRow projection, |a-b| = a+b-2min algebra
with the S seed matmul, exact self term, exp accum_out) is as in the
non-symmetric kernel; see the docstring history in git.  The container's
walrus only allows single-op tensor_scalar on DVE/POOL and requires
DoubleRow matmuls to write at PSUM partition base 0 (hence one PSUM
tile per output row).
"""

import sys
import numpy as np

if "/opt/trn_rl_repo" not in sys.path:
    sys.path.insert(0, "/opt/trn_rl_repo")

NB = 512          # batch rows
NIN = 1024        # n_in
NBF = 64          # n_B
NCD = 16          # n_C
FOUT = NBF * NCD  # 1024 projection features
NCORES = 8
IB = NB // NCORES  # 64 output rows per core
JW = 5 * IB        # 320-column local j window
JA = 4 * IB        # row-sum A range [0, 256)
WSCALE = 64.0      # host multiplies W by this; psum copy divides it out

F8_TILES = (0, 1, 2, 3)   # fp8 min-path (DoubleRow matmuls)
F16_TILES = (4, 5, 6, 7)  # f16 min-path (DVE 4x mode)

# engine per (i%2, tile) for the fp8 min ops ('V' DVE / 'P' POOL)
MIN_ENG = {
    (0, 0): "V", (0, 1): "P", (0, 2): "P", (0, 3): "P",
    (1, 0): "P", (1, 1): "P", (1, 2): "P", (1, 3): "V",
}
JUNK_K = 230.0     # scale for the junk rows (cores 4-7): projects every
                   # feature-sum S to ~230 so exp(-psd) underflows to 0

_CACHE = {}


def _build_program():
    import concourse.bass as bass
    import concourse.tile as tile
    from concourse import mybir
    from contextlib import ExitStack

    f32 = mybir.dt.float32
    f16 = mybir.dt.float16
    f8 = mybir.dt.float8e4
    Alu = mybir.AluOpType
    Act = mybir.ActivationFunctionType
    DR = mybir.MatmulPerfMode.DoubleRow

    nc = bass.Bass()
    x8_d = nc.declare_dram_parameter("x8", [128, 4 * 2 * JW], f8, isOutput=False)
    w8_d = nc.declare_dram_parameter("w8", [128, 8 * 4 * 2 * 128], f8, isOutput=False)
    i8_d = nc.declare_dram_parameter("i8", [128, 2 * 2 * NBF], f8, isOutput=False)
    i16_d = nc.declare_dram_parameter("i16", [128, 4 * NBF], f16, isOutput=False)
    iS8_d = nc.declare_dram_parameter("iS8", [128, 4 * NBF], f8, isOutput=False)
    iS16_d = nc.declare_dram_parameter("iS16", [128, 4 * NBF], f16, isOutput=False)
    stkI_d = nc.declare_dram_parameter("stkI", [NBF, 128], f16, isOutput=False)
    iC8_d = nc.declare_dram_parameter("iC8", [NBF, 2 * NBF], f8, isOutput=False)
    b_d = nc.declare_dram_parameter("b", [FOUT], f32, isOutput=False)
    oA_d = nc.declare_dram_parameter("oA", [NBF, IB], f32, isOutput=True)
    oc_d = nc.declare_dram_parameter("oc", [NBF, JW - IB], f32, isOutput=True)

    with tile.TileContext(nc) as tc, ExitStack() as ctx:
        singles = ctx.enter_context(tc.tile_pool(name="singles", bufs=1))
        scratch = ctx.enter_context(tc.tile_pool(name="scratch", bufs=16))
        epool = ctx.enter_context(tc.tile_pool(name="epool", bufs=4))
        psA = ctx.enter_context(tc.tile_pool(name="psA", bufs=2, space="PSUM"))
        psS = ctx.enter_context(tc.tile_pool(name="psS", bufs=1, space="PSUM"))
        psC = ctx.enter_context(tc.tile_pool(name="psC", bufs=1, space="PSUM"))
        psB = ctx.enter_context(tc.tile_pool(name="psB", bufs=4, space="PSUM"))

        dma = nc.default_dma_engine

        # ---- persistent loads -------------------------------------------
        x8 = singles.tile([128, 4, 2, JW], f8, name="x8", tag="x8")
        x8_r = x8_d.rearrange("p (q m j) -> p q m j", q=4, m=2)
        for q in range(4):
            dma.dma_start(out=x8[:, q], in_=x8_r[:, q])
        w8 = singles.tile([128, 8, 4, 2, 128], f8, name="w8", tag="w8")
        w8_r = w8_d.rearrange("p (t q m f) -> p t q m f", t=8, q=4, m=2)
        for t in range(8):
            dma.dma_start(out=w8[:, t], in_=w8_r[:, t])
        i8 = singles.tile([128, 2, 2, NBF], f8, name="i8", tag="i8")
        dma.dma_start(out=i8, in_=i8_d.rearrange("p (q m b) -> p q m b", q=2, m=2))
        i16 = singles.tile([128, 4, NBF], f16, name="i16", tag="i16")
        dma.dma_start(out=i16, in_=i16_d.rearrange("p (t b) -> p t b", t=4))
        iS8 = singles.tile([128, 4, NBF], f8, name="iS8", tag="iS8")
        dma.dma_start(out=iS8, in_=iS8_d.rearrange("p (t b) -> p t b", t=4))
        iS16 = singles.tile([128, 4, NBF], f16, name="iS16", tag="iS16")
        dma.dma_start(out=iS16, in_=iS16_d.rearrange("p (t b) -> p t b", t=4))
        stkI = singles.tile([NBF, 128], f16, name="stkI", tag="stkI")
        dma.dma_start(out=stkI, in_=stkI_d[:, :])
        iC8 = singles.tile([NBF, 2, NBF], f8, name="iC8", tag="iC8")
        dma.dma_start(out=iC8, in_=iC8_d.rearrange("p (m b) -> p m b", m=2))
        b_sb = singles.tile([128, 8], f32, name="b_sb", tag="b_sb")
        dma.dma_start(out=b_sb, in_=b_d.rearrange("(t p) -> p t", p=128))

        # ---- mT = (x @ W.T)/WSCALE + b  via fp8 DoubleRow ---------------
        mT = [None] * 8
        mC = [None] * 8
        for t in range(8):
            ps = psA.tile([128, JW], f32, name="mps", tag="mps")
            for q in range(4):
                nc.tensor.matmul(
                    ps, lhsT=w8[:, t, q], rhs=x8[:, q],
                    perf_mode=DR, start=(q == 0), stop=(q == 3),
                )
            mt = singles.tile([128, JW], f16, name=f"mT{t}", tag=f"mT{t}")
            nc.scalar.activation(
                out=mt, in_=ps, func=Act.Identity,
                bias=b_sb[:, t : t + 1], scale=1.0 / WSCALE,
            )
            mT[t] = mt
            # f32 upcast of the 64 local columns (scalar operands)
            mc = singles.tile([128, IB], f32, name=f"mC{t}", tag=f"mC{t}")
            nc.vector.tensor_copy(mc, mt[:, 0:IB])
            mC[t] = mc

        # fp8 copies of the fp8-path tiles (S must sum what min will emit)
        m8 = {}
        for t in F8_TILES:
            c8 = singles.tile([128, JW], f8, name=f"m8_{t}", tag=f"m8_{t}")
            nc.vector.tensor_copy(c8, mT[t])
            m8[t] = c8

        # ---- S = sum_c m over min-path tiles  (psum, f32-exact) ---------
        pS = psS.tile([NBF, JW], f32, name="pS", tag="pS")
        for t in F8_TILES:
            nc.tensor.matmul(pS, lhsT=iS8[:, t], rhs=m8[t],
                             start=(t == 0), stop=False)
        for t in F16_TILES:
            nc.tensor.matmul(pS, lhsT=iS16[:, t - 4], rhs=mT[t],
                             start=False, stop=(t == 7))

        negS = singles.tile([NBF, JW], f16, name="negS", tag="negS")
        nc.scalar.activation(out=negS, in_=pS, func=Act.Copy, bias=0.0, scale=-1.0)
        bias64 = singles.tile([NBF, IB], f32, name="bias64", tag="bias64")
        nc.vector.scalar_tensor_tensor(
            out=bias64, in0=pS[:, 0:IB], scalar=-2.0, in1=negS[:, 0:IB],
            op0=Alu.mult, op1=Alu.subtract,
        )

        oA = singles.tile([NBF, IB], f32, name="oA", tag="oA")
        pC = psC.tile([NBF, JW - IB], f32, name="pC", tag="pC")

        # ---- pairwise loop, one row per PSUM tile -----------------------
        E2 = None
        for i in range(IB):
            psd = psB.tile([NBF, JW], f32, name="psd", tag="psd")
            if i % 2 == 0:
                E2 = epool.tile([NBF, 2, JW], f8, name="E2", tag="E2")
            for q in range(2):
                ab8 = scratch.tile([128, 2, JW], f8, name="ab", tag="ab")
                for mm in range(2):
                    t = 2 * q + mm
                    e = (nc.vector if MIN_ENG[(i % 2, t)] == "V"
                         else nc.gpsimd)
                    e.tensor_scalar_min(ab8[:, mm], mT[t], mC[t][:, i : i + 1])
                nc.tensor.matmul(
                    psd, lhsT=i8[:, q], rhs=ab8,
                    perf_mode=DR, start=(q == 0), stop=False,
                )
            for t in F16_TILES:
                ab16 = scratch.tile([128, JW], f16, name="ab16", tag="ab16")
                nc.vector.tensor_scalar_min(ab16, mT[t], mC[t][:, i : i + 1])
                nc.tensor.matmul(
                    psd, lhsT=i16[:, t - 4], rhs=ab16,
                    start=False, stop=False,
                )
            nc.tensor.matmul(psd, lhsT=stkI[:, 0:NBF], rhs=negS,
                             start=False, stop=True)
            nc.scalar.activation(
                out=E2[:, i % 2], in_=psd, func=Act.Exp,
                bias=bias64[:, i : i + 1], scale=1.0,
                accum_out=oA[:, i : i + 1],
            )
            if i % 2 == 1:
                # column sums over j in [64, 320): DoubleRow accumulates
                # both rows' E at once
                nc.tensor.matmul(pC, lhsT=iC8, rhs=E2[:, :, IB:JW],
                                 perf_mode=DR,
                                 start=(i == 1), stop=(i == IB - 1))

        ocs = singles.tile([NBF, JW - IB], f32, name="ocs", tag="ocs")
        nc.scalar.activation(out=ocs, in_=pC, func=Act.Copy, bias=0.0, scale=1.0)

        dma.dma_start(out=oA_d[:, :], in_=oA)
        dma.dma_start(out=oc_d[:, :], in_=ocs)

    _split_multi_waits(nc, mybir)
    return nc


def _split_multi_waits(nc, mybir):
    """Hoist multi-waits onto single-wait NoOps (walrus limitation)."""
    f = nc.m.functions[0]
    n_split = 0
    for blk in f.blocks:
        idx = 0
        while idx < len(blk.instructions):
            inst = blk.instructions[idx]
            si = inst.sync_info
            waits = list(si.on_wait) if si is not None and si.on_wait else []
            if len(waits) > 1:
                bysem = {}
                for w in waits:
                    k = w.id
                    if k not in bysem or (w.wait_value or 0) > (
                        bysem[k].wait_value or 0
                    ):
                        bysem[k] = w
                waits = list(bysem.values())
                for w in waits[:-1]:
                    nop = mybir.InstNoOp(
                        name=nc.get_next_instruction_name(), ins=[], outs=[]
                    )
                    nop.engine = inst.engine
                    nop.sync_info = mybir.SyncInfo(on_wait=[w], on_update=[])
                    blk.instructions.insert(idx, nop)
                    idx += 1
                    n_split += 1
                si.on_wait = [waits[-1]]
            idx += 1
    return n_split


def _get_program():
    if "nc" not in _CACHE:
        _CACHE["nc"] = _build_program()
    return _CACHE["nc"]


def make_in_maps(x, W, b):
    import ml_dtypes

    f8 = ml_dtypes.float8_e4m3
    x = np.ascontiguousarray(x, dtype=np.float32)
    W = np.ascontiguousarray(W, dtype=np.float32)
    b = np.ascontiguousarray(b, dtype=np.float32)

    wT = np.ascontiguousarray(W.T * WSCALE).astype(f8)          # [1024 k, 1024 f]
    w8 = wT.reshape(4, 2, 128, 8, 128)                           # [q, m, p, t, f]
    w8 = np.ascontiguousarray(w8.transpose(2, 3, 0, 1, 4)).reshape(128, -1)

    ind = np.zeros((8, 128, NBF), dtype=np.float32)
    ch = np.arange(FOUT).reshape(8, 128)
    t_, p_ = np.meshgrid(np.arange(8), np.arange(128), indexing="ij")
    ind[t_, p_, ch // NCD] = 1.0
    i8 = np.ascontiguousarray(
        (2.0 * ind[:4]).reshape(2, 2, 128, NBF).transpose(2, 0, 1, 3)
    ).astype(f8).reshape(128, -1)
    i16 = np.ascontiguousarray(
        (2.0 * ind[4:]).transpose(1, 0, 2)
    ).astype(np.float16).reshape(128, -1)
    iS8 = np.ascontiguousarray(ind[:4].transpose(1, 0, 2)).astype(f8).reshape(128, -1)
    iS16 = np.ascontiguousarray(ind[4:].transpose(1, 0, 2)).astype(np.float16).reshape(128, -1)
    stkI = np.concatenate([np.eye(NBF), np.eye(NBF)], axis=1).astype(np.float16)
    iC8 = np.concatenate([np.eye(NBF), np.eye(NBF)], axis=1).astype(f8)

    U = W.reshape(NBF, NCD, NIN).sum(1)
    xs = (U.T @ np.linalg.solve(U @ U.T, np.full(NBF, 1000.0))).astype(np.float32)
    xs *= JUNK_K / np.abs(xs).max()

    in_maps = []
    for c in range(NCORES):
        xr = np.roll(x, -IB * c, axis=0)[0:JW].copy()            # [320, 1024]
        if c >= 4:
            xr[JA:JW] = xs[None, :]   # junk rows: E == 0 exactly
        xT = np.ascontiguousarray(xr.T).astype(f8)               # [1024 k, 320 j]
        x8 = xT.reshape(4, 2, 128, JW)                            # [q, m, p, j]
        x8 = np.ascontiguousarray(x8.transpose(2, 0, 1, 3)).reshape(128, -1)
        in_maps.append({
            "x8": x8, "w8": w8, "i8": i8, "i16": i16,
            "iS8": iS8, "iS16": iS16, "stkI": stkI, "iC8": iC8, "b": b,
        })
    return in_maps


def assemble_o(results):
    """results[c] -> dict with 'oA' [64,64], 'oc' [64,256]."""
    o_full = np.zeros((NB, NBF), dtype=np.float64)
    for c in range(NCORES):
        o_full[IB * c : IB * (c + 1)] += np.asarray(results[c]["oA"],
                                                    dtype=np.float64).T
        oc = np.asarray(results[c]["oc"], dtype=np.float64)
        g = (IB * c + IB + np.arange(JW - IB)) % NB
        o_full[g] += oc.T
    return o_full.astype(np.float32)


def kernel(x, W, b):
    from concourse.bass_utils import run_bass_kernel_spmd

    x = np.ascontiguousarray(x, dtype=np.float32)
    nc = _get_program()
    in_maps = make_in_maps(x, W, b)

    res = run_bass_kernel_spmd(nc, in_maps, list(range(NCORES)), trace=False)
    _CACHE["last_results"] = res

    o_full = assemble_o(res.results)
    return np.concatenate([x, o_full], axis=1)
